# revision 32
# baseline (speedup 1.0000x reference)
"""Trainium2 Bass kernel for nn_Jointer: per-sample masked cosine-similarity.

out[b] = relu(l2norm(source[b]) @ l2norm(target[b]).T) * (mask_src[b] outer mask_tar[b])

The masks kill ~75% of the output (ragged_sequence): only valid source rows x
valid target cols are nonzero. Host side gathers the valid tokens per sample,
l2-normalizes, pre-transposes to [D, tokens] and casts to bf16; the device
computes just the compact relu(sim) block (bf16 in/out, f32 PSUM accumulate);
host scatters the compact block back into the zero-filled full f32 output.
Per core that is ~0.6 MB in + ~2.7 MB out of HBM traffic instead of 18.8 MB
dense f32.

Device: raw bass (no TileContext — its prologue/teardown semaphore walk costs
~10us). Chunk-granular pipeline per core:
- chunked input loads on the two HWDGE rings (the first matmul starts as soon
  as the first chunks' completion semaphores land)
- 27 matmuls (384-wide) into one 8-bank PSUM tensor, chunk k -> 512-aligned
  slot k%8 (8-deep rotation so the PE never stalls on drains)
- relu+bf16 drains in PAIRS of chunks (strided [128,2,384] PSUM read) to
  amortize the ~150-cycle fixed cost per ACT/DVE op, alternating engines
- row-level output DMAs from Sync (split on first/last rows for ramp/tail)
All cross-DMA dependencies use exact per-object semaphores (DMA completions
across separate dma_starts are unordered).

Sharding: data-parallel over batch B=8 -> one sample per NeuronCore.
"""

import contextlib

import numpy as np
import ml_dtypes

import concourse.bass as bass
from concourse import bacc
import concourse.mybir as mybir
from concourse.bass_utils import run_bass_kernel_spmd

F32 = mybir.dt.float32
BF16 = mybir.dt.bfloat16
AF = mybir.ActivationFunctionType

P = 128  # partitions (= feature dim D = contraction dim)
BANK = 512  # PSUM bank, fp32 elements
NSLOT = 8  # PSUM slots (one bank each)
EPS = 1e-12


def _chunks(n, cap=512):
    """Split n (multiple of 128) into near-equal multiples of 128, each <= cap."""
    k = -(-n // cap)
    base = n // k // P * P
    rem = (n - base * k) // P
    widths = [base + P if i < rem else base for i in range(k)]
    out, pos = [], 0
    for w in widths:
        out.append((pos, w))
        pos += w
    return out


def build_nc(NS, NT) -> bass.Bass:
    nc = bacc.Bacc(trn_type="TRN2")

    sT = nc.dram_tensor("sT", [P, NS], BF16, kind="ExternalInput")
    tT = nc.dram_tensor("tT", [P, NT], BF16, kind="ExternalInput")
    out = nc.dram_tensor("out", [NS, NT], BF16, kind="ExternalOutput")
    out_r = out.rearrange("(m p) n -> m p n", p=P)
    sT_r = sT.rearrange("p n -> p n")
    tT_r = tT.rearrange("p n -> p n")

    MB = NS // P
    ch = _chunks(NT)
    NCH = len(ch)
    NK = MB * NCH
    # One SBUF output buffer per row: no ob recycling, so drains never wait
    # on output-DMA receipts (the ~1.5us HBM write-receipt latency was the
    # binding tail constraint with a small rotation).
    NOB = MB

    def row(k):
        return k // NCH

    def width(k):
        return ch[k % NCH][1]

    def obpos(k):
        return (row(k) % NOB) * NT + ch[k % NCH][0]

    # Group chunks into drain pairs where PSUM slots are adjacent, widths
    # equal, and the ob destinations contiguous; singles otherwise. The last
    # row drains as singles: a pair would gate its DMA on the second-to-last
    # matmul, lengthening the tail.
    groups = []  # list of [k] or [k, k+1]
    k = 0
    while k < NK:
        if (
            k + 1 < NK
            and row(k) < MB - 1
            and width(k) == width(k + 1)
            and obpos(k + 1) == obpos(k) + width(k)
            and (k % NSLOT) + 1 == (k + 1) % NSLOT
        ):
            groups.append([k, k + 1])
            k += 2
        else:
            groups.append([k])
            k += 1
    NG = len(groups)
    group_of = {}
    for gi, g in enumerate(groups):
        for c in g:
            group_of[c] = gi

    def g_eng(gi):  # 0 = ACT, 1 = DVE
        return gi % 2

    def g_cnt(gi):  # completions on gi's engine once groups 0..gi are done
        return gi // 2 + 1

    # out-DMA plan: rows 0 and MB-1 split at drain-group boundaries within
    # the row; other rows one DMA. Element spans per row (start, end, gate
    # group)  — gate = last group covering a chunk in the span.
    def row_spans(m):
        lo_k, hi_k = m * NCH, (m + 1) * NCH - 1
        if m in (0, MB - 1) and NCH > 1:
            spans = []
            cur = lo_k
            while cur <= hi_k:
                g = groups[group_of[cur]]
                last = min(g[-1], hi_k)
                spans.append((cur, last))
                cur = last + 1
            return spans
        return [(lo_k, hi_k)]

    n_dmas = sum(len(row_spans(m)) for m in range(MB))

    with contextlib.ExitStack() as stack:
        ec = stack.enter_context
        s_ins0 = ec(nc.semaphore("s_ins0"))
        s_ins1 = ec(nc.semaphore("s_ins1"))
        s_int = [ec(nc.semaphore(f"s_int{i}")) for i in range(NCH)]
        s_mm = ec(nc.semaphore("s_mm"))
        s_pa = ec(nc.semaphore("s_pa"))
        s_pv = ec(nc.semaphore("s_pv"))
        s_out = ec(nc.semaphore("s_out"))
        sT_sb = ec(nc.sbuf_tensor("sT_sb", [P, NS], BF16))
        tT_sb = ec(nc.sbuf_tensor("tT_sb", [P, NT], BF16))
        ob = ec(nc.sbuf_tensor("ob", [P, NOB * NT], BF16))
        ps_all = ec(nc.psum_tensor("ps_all", [P, NSLOT * BANK], F32))

        s_eng = [s_pa, s_pv]

        def wait_groups_done(eng, gi, prev):
            """Wait until groups 0..gi are all drained (dedupe via prev)."""
            for e in (0, 1):
                last = gi if g_eng(gi) == e else gi - 1
                if last < 0:
                    continue
                # count of engine-e groups among 0..gi
                cnt = sum(1 for j in range(gi + 1) if g_eng(j) == e)
                if cnt > prev[e]:
                    prev[e] = cnt
                    eng.wait_ge(s_eng[e], cnt)

        # Output-DMA issue costs ~600ns of DIRECT2D per dma_start on the
        # issuing sequencer; spread rows across both HWDGE rings: even rows
        # on Sync, odd rows (and the final span) on Scalar, interleaved with
        # its drains.
        def emit_row_dma(eng, m, spans, prev):
            sl = (m % NOB) * NT
            for lo_k, hi_k in spans:
                wait_groups_done(eng, group_of[hi_k], prev)
                a = ch[lo_k % NCH][0]
                b = ch[hi_k % NCH][0] + width(hi_k)
                eng.dma_start(
                    out_r[m][:, a:b], ob[:, sl + a : sl + b]
                ).then_inc(s_out, 16)

        def dma_plan(m):
            """(engine, spans): 'sync' even rows, 'gpsimd' (idle SWDGE ring)
            odd rows; the last row's final span goes to scalar (HWDGE, idle
            by then — its completion gates the end of the kernel)."""
            spans = row_spans(m)
            if m == MB - 1 and MB >= 2 and len(spans) > 1:
                return [("sync" if m % 2 == 0 else "gpsimd", spans[:-1]),
                        ("scalar", spans[-1:])]
            return [("sync" if m % 2 == 0 else "gpsimd", spans)]

        scalar_rows = {}  # gate group index -> list of (m, spans)
        sync_rows = []
        gpsimd_rows = []
        for m in range(MB):
            for eng_name, spans in dma_plan(m):
                if eng_name == "sync":
                    sync_rows.append((m, spans))
                elif eng_name == "gpsimd":
                    gpsimd_rows.append((m, spans))
                else:
                    gate = group_of[spans[-1][1]]
                    scalar_rows.setdefault(gate, []).append((m, spans))

        with nc.Block() as block:

            @block.sync
            def _(sync):
                sync.dma_start(sT_sb[:, :P], sT_r[:, :P]).then_inc(s_ins0, 16)
                sync.dma_start(sT_sb[:, P:], sT_r[:, P:]).then_inc(s_ins1, 16)
                prev = [0, 0]
                for m, spans in sync_rows:
                    emit_row_dma(sync, m, spans, prev)
                sync.wait_ge(s_out, 16 * n_dmas)

            def drain_stream(eng_idx, eng, dma_by_gate=None, prev=None):
                pm = [0]
                for gi in range(NG):
                    if g_eng(gi) == eng_idx:
                        g = groups[gi]
                        if g[-1] + 1 > pm[0]:
                            pm[0] = g[-1] + 1
                            eng.wait_ge(s_mm, pm[0])
                        w = width(g[0])
                        slot = g[0] % NSLOT
                        if len(g) == 2:
                            src = bass.AP(
                                ps_all,
                                slot * BANK,
                                [[NSLOT * BANK, P], [BANK, 2], [1, w]],
                            )
                            dst = ob[:, obpos(g[0]) : obpos(g[0]) + 2 * w]
                        else:
                            src = ps_all[:, slot * BANK : slot * BANK + w]
                            dst = ob[:, obpos(g[0]) : obpos(g[0]) + w]
                        if eng_idx == 0:
                            eng.activation(
                                out=dst, in_=src, func=AF.Relu
                            ).then_inc(s_pa, 1)
                        else:
                            eng.tensor_scalar_max(
                                out=dst, in0=src, scalar1=0.0
                            ).then_inc(s_pv, 1)
                    if dma_by_gate and gi in dma_by_gate:
                        for m, spans in dma_by_gate[gi]:
                            emit_row_dma(eng, m, spans, prev)

            @block.scalar
            def _(scalar):
                n0, w0 = ch[0]
                scalar.dma_start(
                    tT_sb[:, : n0 + w0], tT_r[:, : n0 + w0]
                ).then_inc(s_int[0], 16)
                if NCH > 1:
                    # remaining tT chunks in one DMA (completion sems of
                    # separate DMAs are unordered, so one sem covers c1..)
                    rest = n0 + w0
                    scalar.dma_start(
                        tT_sb[:, rest:], tT_r[:, rest:]
                    ).then_inc(s_int[NCH - 1], 16)
                drain_stream(0, scalar, scalar_rows, [0, 0])

            @block.vector
            def _(vector):
                drain_stream(1, vector)

            if gpsimd_rows:

                @block.gpsimd
                def _(gpsimd):
                    prev = [0, 0]
                    for m, spans in gpsimd_rows:
                        emit_row_dma(gpsimd, m, spans, prev)

            @block.tensor
            def _(tensor):
                prev = [0, 0]
                for k in range(NK):
                    m, ci = k // NCH, k % NCH
                    n0, w = ch[ci]
                    if k == ci:  # first row: tT chunk ci needed
                        # c0 loads alone; c1.. arrive via one merged DMA
                        tensor.wait_ge(s_int[0] if ci == 0 else s_int[NCH - 1], 16)
                        if k == 0:
                            tensor.wait_ge(s_ins0, 16)
                    if k == NCH:  # second row: rest of sT needed
                        tensor.wait_ge(s_ins1, 16)
                    if k >= NSLOT:  # slot reuse: chunk k-NSLOT drained
                        wait_groups_done(tensor, group_of[k - NSLOT], prev)
                    slot = k % NSLOT
                    tensor.matmul(
                        ps_all[:, slot * BANK : slot * BANK + w],
                        sT_sb[:, m * P : (m + 1) * P],
                        tT_sb[:, n0 : n0 + w],
                        start=True,
                        stop=True,
                    ).then_inc(s_mm, 1)

        nc.compile()
    return nc


_NC_CACHE = {}


def _get_nc(NS, NT):
    key = (NS, NT)
    if key not in _NC_CACHE:
        _NC_CACHE[key] = build_nc(NS, NT)
    return _NC_CACHE[key]


def _pad128(n):
    return max(P, -(-n // P) * P)


def kernel(source, target, mask_src, mask_tar, **run_kwargs):
    source = np.asarray(source, dtype=np.float32)
    target = np.asarray(target, dtype=np.float32)
    mask_src = np.asarray(mask_src).astype(bool)
    mask_tar = np.asarray(mask_tar).astype(bool)
    B, S, D = source.shape
    T = target.shape[1]

    idx_s = [np.flatnonzero(mask_src[b]) for b in range(B)]
    idx_t = [np.flatnonzero(mask_tar[b]) for b in range(B)]
    NS = _pad128(max(len(i) for i in idx_s))
    NT = _pad128(max(len(i) for i in idx_t))

    in_maps = []
    for b in range(B):
        s = source[b][idx_s[b]]
        t = target[b][idx_t[b]]
        s = s / np.maximum(np.linalg.norm(s, axis=1, keepdims=True), EPS)
        t = t / np.maximum(np.linalg.norm(t, axis=1, keepdims=True), EPS)
        sTb = np.zeros((P, NS), dtype=ml_dtypes.bfloat16)
        tTb = np.zeros((P, NT), dtype=ml_dtypes.bfloat16)
        sTb[:, : len(idx_s[b])] = s.T.astype(ml_dtypes.bfloat16)
        tTb[:, : len(idx_t[b])] = t.T.astype(ml_dtypes.bfloat16)
        in_maps.append({"sT": sTb, "tT": tTb})

    nc = _get_nc(NS, NT)
    res = run_bass_kernel_spmd(nc, in_maps, core_ids=list(range(B)), **run_kwargs)

    full = np.zeros((B, S, T), dtype=np.float32)
    for b in range(B):
        oc = np.asarray(res.results[b]["out"]).astype(np.float32)
        ns, nt = len(idx_s[b]), len(idx_t[b])
        if ns and nt:
            full[b][np.ix_(idx_s[b], idx_t[b])] = oc[:ns, :nt]
    if run_kwargs.get("trace"):
        kernel.last_results = res
    return full


# revision 33
# speedup vs baseline: 1.0667x; 1.0667x over previous
"""Trainium2 Bass kernel for nn_Jointer: per-sample masked cosine-similarity.

out[b] = relu(l2norm(source[b]) @ l2norm(target[b]).T) * (mask_src[b] outer mask_tar[b])

The masks kill ~75% of the output (ragged_sequence): only valid source rows x
valid target cols are nonzero. Host side gathers the valid tokens per sample,
l2-normalizes, pre-transposes to [D, tokens] and casts to bf16; the device
computes just the compact relu(sim) block (bf16 in/out, f32 PSUM accumulate);
host scatters the compact block back into the zero-filled full f32 output.
Per core that is ~0.6 MB in + ~2.7 MB out of HBM traffic instead of 18.8 MB
dense f32.

Device: raw bass (no TileContext — its prologue/teardown semaphore walk costs
~10us). Chunk-granular pipeline per core:
- chunked input loads on the two HWDGE rings (the first matmul starts as soon
  as the first chunks' completion semaphores land)
- 27 matmuls (384-wide) into one 8-bank PSUM tensor, chunk k -> 512-aligned
  slot k%8 (8-deep rotation so the PE never stalls on drains)
- relu+bf16 drains in PAIRS of chunks (strided [128,2,384] PSUM read) to
  amortize the ~150-cycle fixed cost per ACT/DVE op, alternating engines
- row-level output DMAs from Sync (split on first/last rows for ramp/tail)
All cross-DMA dependencies use exact per-object semaphores (DMA completions
across separate dma_starts are unordered).

Sharding: data-parallel over batch B=8 -> one sample per NeuronCore.
"""

import contextlib

import numpy as np
import ml_dtypes

import concourse.bass as bass
from concourse import bacc
import concourse.mybir as mybir
from concourse.bass_utils import run_bass_kernel_spmd

F32 = mybir.dt.float32
BF16 = mybir.dt.bfloat16
AF = mybir.ActivationFunctionType

P = 128  # partitions (= feature dim D = contraction dim)
BANK = 512  # PSUM bank, fp32 elements
NSLOT = 8  # PSUM slots (one bank each)
EPS = 1e-12


def _chunks(n, cap=512):
    """Split n (multiple of 128) into near-equal multiples of 128, each <= cap."""
    k = -(-n // cap)
    base = n // k // P * P
    rem = (n - base * k) // P
    widths = [base + P if i < rem else base for i in range(k)]
    out, pos = [], 0
    for w in widths:
        out.append((pos, w))
        pos += w
    return out


def build_nc(NS, NT) -> bass.Bass:
    nc = bacc.Bacc(trn_type="TRN2")

    sT = nc.dram_tensor("sT", [P, NS], BF16, kind="ExternalInput")
    tT = nc.dram_tensor("tT", [P, NT], BF16, kind="ExternalInput")
    out = nc.dram_tensor("out", [NS, NT], BF16, kind="ExternalOutput")
    out_r = out.rearrange("(m p) n -> m p n", p=P)
    sT_r = sT.rearrange("p n -> p n")
    tT_r = tT.rearrange("p n -> p n")

    MB = NS // P
    ch = _chunks(NT)
    NCH = len(ch)
    NK = MB * NCH
    # One SBUF output buffer per row: no ob recycling, so drains never wait
    # on output-DMA receipts (the ~1.5us HBM write-receipt latency was the
    # binding tail constraint with a small rotation).
    NOB = MB

    def row(k):
        return k // NCH

    def width(k):
        return ch[k % NCH][1]

    def obpos(k):
        return (row(k) % NOB) * NT + ch[k % NCH][0]

    # Group chunks into drain pairs where PSUM slots are adjacent, widths
    # equal, and the ob destinations contiguous; singles otherwise. The last
    # row drains as singles: a pair would gate its DMA on the second-to-last
    # matmul, lengthening the tail.
    groups = []  # list of [k] or [k, k+1]
    k = 0
    while k < NK:
        if (
            k + 1 < NK
            and row(k) < MB - 1
            and width(k) == width(k + 1)
            and obpos(k + 1) == obpos(k) + width(k)
            and (k % NSLOT) + 1 == (k + 1) % NSLOT
        ):
            groups.append([k, k + 1])
            k += 2
        else:
            groups.append([k])
            k += 1
    NG = len(groups)
    group_of = {}
    for gi, g in enumerate(groups):
        for c in g:
            group_of[c] = gi

    def g_eng(gi):  # 0 = ACT, 1 = DVE
        return gi % 2

    def g_cnt(gi):  # completions on gi's engine once groups 0..gi are done
        return gi // 2 + 1

    # out-DMA plan: rows 0 and MB-1 split at drain-group boundaries within
    # the row; other rows one DMA. Element spans per row (start, end, gate
    # group)  — gate = last group covering a chunk in the span.
    def row_spans(m):
        lo_k, hi_k = m * NCH, (m + 1) * NCH - 1
        if m in (0, MB - 1) and NCH > 1:
            spans = []
            cur = lo_k
            while cur <= hi_k:
                g = groups[group_of[cur]]
                last = min(g[-1], hi_k)
                spans.append((cur, last))
                cur = last + 1
            return spans
        return [(lo_k, hi_k)]

    n_dmas = sum(len(row_spans(m)) for m in range(MB))

    with contextlib.ExitStack() as stack:
        ec = stack.enter_context
        s_ins0 = ec(nc.semaphore("s_ins0"))
        s_ins1 = ec(nc.semaphore("s_ins1"))
        s_int = [ec(nc.semaphore(f"s_int{i}")) for i in range(NCH)]
        s_mm = ec(nc.semaphore("s_mm"))
        s_pa = ec(nc.semaphore("s_pa"))
        s_pv = ec(nc.semaphore("s_pv"))
        s_out = ec(nc.semaphore("s_out"))
        sT_sb = ec(nc.sbuf_tensor("sT_sb", [P, NS], BF16))
        tT_sb = ec(nc.sbuf_tensor("tT_sb", [P, NT], BF16))
        ob = ec(nc.sbuf_tensor("ob", [P, NOB * NT], BF16))
        ps_all = ec(nc.psum_tensor("ps_all", [P, NSLOT * BANK], F32))

        s_eng = [s_pa, s_pv]

        def wait_groups_done(eng, gi, prev):
            """Wait until groups 0..gi are all drained (dedupe via prev)."""
            for e in (0, 1):
                last = gi if g_eng(gi) == e else gi - 1
                if last < 0:
                    continue
                # count of engine-e groups among 0..gi
                cnt = sum(1 for j in range(gi + 1) if g_eng(j) == e)
                if cnt > prev[e]:
                    prev[e] = cnt
                    eng.wait_ge(s_eng[e], cnt)

        # Output-DMA issue costs ~600ns of DIRECT2D per dma_start on the
        # issuing sequencer; spread rows across both HWDGE rings: even rows
        # on Sync, odd rows (and the final span) on Scalar, interleaved with
        # its drains.
        def emit_row_dma(eng, m, spans, prev):
            sl = (m % NOB) * NT
            for lo_k, hi_k in spans:
                wait_groups_done(eng, group_of[hi_k], prev)
                a = ch[lo_k % NCH][0]
                b = ch[hi_k % NCH][0] + width(hi_k)
                eng.dma_start(
                    out_r[m][:, a:b], ob[:, sl + a : sl + b]
                ).then_inc(s_out, 16)

        def dma_plan(m):
            """(engine, spans): rows on Sync; the last row's final span goes
            to scalar (HWDGE, idle by then) so the kernel-ending DMA does
            not queue behind Sync's previous DIRECT2D issue. (Spreading row
            DMAs onto the scalar or gpsimd rings measured slower: issue
            cost throttles ACT's drains / SWDGE latency.)"""
            spans = row_spans(m)
            if m == MB - 1 and MB >= 2 and len(spans) > 1:
                return [("sync", spans[:-1]), ("scalar", spans[-1:])]
            return [("sync", spans)]

        scalar_rows = {}  # gate group index -> list of (m, spans)
        sync_rows = []
        gpsimd_rows = []
        for m in range(MB):
            for eng_name, spans in dma_plan(m):
                if eng_name == "sync":
                    sync_rows.append((m, spans))
                elif eng_name == "gpsimd":
                    gpsimd_rows.append((m, spans))
                else:
                    gate = group_of[spans[-1][1]]
                    scalar_rows.setdefault(gate, []).append((m, spans))

        with nc.Block() as block:

            @block.sync
            def _(sync):
                sync.dma_start(sT_sb[:, :P], sT_r[:, :P]).then_inc(s_ins0, 16)
                sync.dma_start(sT_sb[:, P:], sT_r[:, P:]).then_inc(s_ins1, 16)
                prev = [0, 0]
                for m, spans in sync_rows:
                    emit_row_dma(sync, m, spans, prev)
                sync.wait_ge(s_out, 16 * n_dmas)

            def drain_stream(eng_idx, eng, dma_by_gate=None, prev=None):
                pm = [0]
                for gi in range(NG):
                    if g_eng(gi) == eng_idx:
                        g = groups[gi]
                        if g[-1] + 1 > pm[0]:
                            pm[0] = g[-1] + 1
                            eng.wait_ge(s_mm, pm[0])
                        w = width(g[0])
                        slot = g[0] % NSLOT
                        if len(g) == 2:
                            src = bass.AP(
                                ps_all,
                                slot * BANK,
                                [[NSLOT * BANK, P], [BANK, 2], [1, w]],
                            )
                            dst = ob[:, obpos(g[0]) : obpos(g[0]) + 2 * w]
                        else:
                            src = ps_all[:, slot * BANK : slot * BANK + w]
                            dst = ob[:, obpos(g[0]) : obpos(g[0]) + w]
                        if eng_idx == 0:
                            eng.activation(
                                out=dst, in_=src, func=AF.Relu
                            ).then_inc(s_pa, 1)
                        else:
                            eng.tensor_scalar_max(
                                out=dst, in0=src, scalar1=0.0
                            ).then_inc(s_pv, 1)
                    if dma_by_gate and gi in dma_by_gate:
                        for m, spans in dma_by_gate[gi]:
                            emit_row_dma(eng, m, spans, prev)

            @block.scalar
            def _(scalar):
                n0, w0 = ch[0]
                scalar.dma_start(
                    tT_sb[:, : n0 + w0], tT_r[:, : n0 + w0]
                ).then_inc(s_int[0], 16)
                if NCH > 1:
                    # remaining tT chunks in one DMA (completion sems of
                    # separate DMAs are unordered, so one sem covers c1..)
                    rest = n0 + w0
                    scalar.dma_start(
                        tT_sb[:, rest:], tT_r[:, rest:]
                    ).then_inc(s_int[NCH - 1], 16)
                drain_stream(0, scalar, scalar_rows, [0, 0])

            @block.vector
            def _(vector):
                drain_stream(1, vector)

            if gpsimd_rows:

                @block.gpsimd
                def _(gpsimd):
                    prev = [0, 0]
                    for m, spans in gpsimd_rows:
                        emit_row_dma(gpsimd, m, spans, prev)

            @block.tensor
            def _(tensor):
                prev = [0, 0]
                for k in range(NK):
                    m, ci = k // NCH, k % NCH
                    n0, w = ch[ci]
                    if k == ci:  # first row: tT chunk ci needed
                        # c0 loads alone; c1.. arrive via one merged DMA
                        tensor.wait_ge(s_int[0] if ci == 0 else s_int[NCH - 1], 16)
                        if k == 0:
                            tensor.wait_ge(s_ins0, 16)
                    if k == NCH:  # second row: rest of sT needed
                        tensor.wait_ge(s_ins1, 16)
                    if k >= NSLOT:  # slot reuse: chunk k-NSLOT drained
                        wait_groups_done(tensor, group_of[k - NSLOT], prev)
                    slot = k % NSLOT
                    tensor.matmul(
                        ps_all[:, slot * BANK : slot * BANK + w],
                        sT_sb[:, m * P : (m + 1) * P],
                        tT_sb[:, n0 : n0 + w],
                        start=True,
                        stop=True,
                    ).then_inc(s_mm, 1)

        nc.compile()
    return nc


_NC_CACHE = {}


def _get_nc(NS, NT):
    key = (NS, NT)
    if key not in _NC_CACHE:
        _NC_CACHE[key] = build_nc(NS, NT)
    return _NC_CACHE[key]


def _pad128(n):
    return max(P, -(-n // P) * P)


def kernel(source, target, mask_src, mask_tar, **run_kwargs):
    source = np.asarray(source, dtype=np.float32)
    target = np.asarray(target, dtype=np.float32)
    mask_src = np.asarray(mask_src).astype(bool)
    mask_tar = np.asarray(mask_tar).astype(bool)
    B, S, D = source.shape
    T = target.shape[1]

    idx_s = [np.flatnonzero(mask_src[b]) for b in range(B)]
    idx_t = [np.flatnonzero(mask_tar[b]) for b in range(B)]
    NS = _pad128(max(len(i) for i in idx_s))
    NT = _pad128(max(len(i) for i in idx_t))

    in_maps = []
    for b in range(B):
        s = source[b][idx_s[b]]
        t = target[b][idx_t[b]]
        s = s / np.maximum(np.linalg.norm(s, axis=1, keepdims=True), EPS)
        t = t / np.maximum(np.linalg.norm(t, axis=1, keepdims=True), EPS)
        sTb = np.zeros((P, NS), dtype=ml_dtypes.bfloat16)
        tTb = np.zeros((P, NT), dtype=ml_dtypes.bfloat16)
        sTb[:, : len(idx_s[b])] = s.T.astype(ml_dtypes.bfloat16)
        tTb[:, : len(idx_t[b])] = t.T.astype(ml_dtypes.bfloat16)
        in_maps.append({"sT": sTb, "tT": tTb})

    nc = _get_nc(NS, NT)
    res = run_bass_kernel_spmd(nc, in_maps, core_ids=list(range(B)), **run_kwargs)

    full = np.zeros((B, S, T), dtype=np.float32)
    for b in range(B):
        oc = np.asarray(res.results[b]["out"]).astype(np.float32)
        ns, nt = len(idx_s[b]), len(idx_t[b])
        if ns and nt:
            full[b][np.ix_(idx_s[b], idx_t[b])] = oc[:ns, :nt]
    if run_kwargs.get("trace"):
        kernel.last_results = res
    return full
